# revision 1
# baseline (speedup 1.0000x reference)
"""Trainium2 Bass kernel for BatchAllTripletWithClustersLossSemiHard (v6).

Strategy (data-parallel over anchors, 8 cores):
  Pairs (i,j) with equal labels are enumerated per core (anchors greedily
  balanced), SORTED BY j so each 128-pair chunk's j-values fall in a <=64
  column window.  Per pair-row p over k=0..B-1:
      z[p,k] = w_p*(1 + V[i_p,k] - V[i_p,j_p]) + madd[p,k]
  with V[i,k] = 2*x_i.x_k - |x_k|^2.  madd in {0, -4096} (fp8e5)
  dead-masks the semi-hard-rank-excluded k plus k==i and k==j, so no
  host-side corrections are needed.

  Device pipeline per chunk: PSUM accumulates selW.V (float32r matmul)
  + I.madd (fp8 matmul); a tiny DVE STT over the 64-col j-window
  extracts e=w*V_ij-4096; cvec=(w-4096)-e (Pool TT) biases a ScalarE
  relu activation that writes bf16 relu values and row-sums into sacc.
  Counts come from batched DVE is_gt passes over the bf16 relu outputs.
  All inputs are host-prepacked into 4 flat DMAs (one per queue, in
  needed-first order); PE is kept busy with warmup matmuls so the
  p-state ramp reaches full clock before the chunk matmuls.  Raw
  [128,n] partials are DMA'd out; the host does the final reduction.
"""

import numpy as np
import ml_dtypes

import concourse.bass as bass
import concourse.tile as tile
from concourse import bacc, mybir
from concourse.bass_utils import run_bass_kernel_spmd

EPS = 1e-8
NEG = -4096.0
B, D, NCORES = 384, 512, 8
P = 128
KW = 64
F32 = mybir.dt.float32
F32R = mybir.dt.float32r
BF = mybir.dt.bfloat16
F8 = mybir.dt.float8e5


def _host_prep(labels, clusters, weights):
    labels = np.asarray(labels).astype(np.int64)
    clusters = np.asarray(clusters).astype(np.int64)
    weights = np.asarray(weights).astype(np.float32)

    leq = labels[None, :] == labels[:, None]
    rank = np.cumsum(leq.astype(np.int64), axis=1) - 1
    first = leq & (rank % 2 == 1)
    second = leq & (rank % 2 == 0)
    pbase = ~first
    qbase = ~second

    npos = leq.sum(1) - 1
    order = np.argsort(-npos, kind="stable")
    core_anchors = [[] for _ in range(NCORES)]
    core_load = [0] * NCORES
    for i in order:
        c = int(np.argmin(core_load))
        core_anchors[c].append(int(i))
        core_load[c] += int(npos[i])
    MA = max(len(a) for a in core_anchors)

    all_pairs = []
    for c in range(NCORES):
        pairs = []
        for il, i in enumerate(core_anchors[c]):
            for j in np.where(leq[i])[0]:
                if j != i:
                    pairs.append((il, i, int(j)))
        pairs.sort(key=lambda t: t[2])  # j-sorted => narrow windows
        all_pairs.append(pairs)
    NP = ((max(len(p) for p in all_pairs) + P - 1) // P) * P
    NCH = NP // P

    # shared j-window starts (SPMD program must be identical across cores)
    kw0s = []
    for ch in range(NCH):
        lo = min(min((t[2] for t in ap[ch * P:(ch + 1) * P]), default=0)
                 for ap in all_pairs)
        kw0s.append(min(lo, B - KW))
    for c in range(NCORES):
        for ch in range(NCH):
            chunk = all_pairs[c][ch * P:(ch + 1) * P]
            if chunk:
                assert max(t[2] for t in chunk) < kw0s[ch] + KW, \
                    "shared window overflow"

    tables = []
    for c in range(NCORES):
        pairs = all_pairs[c]
        sel = np.zeros((MA, NP), np.float32)
        wc = np.full((P, NCH), NEG, np.float32)   # (w + NEG), [p, ch] packed
        madd = np.full((P, NCH, B), NEG, np.float32)
        jwin = np.zeros((P, NCH, KW), np.float32)
        for ch in range(NCH):
            k0 = kw0s[ch]
            for r, (il, i, j) in enumerate(pairs[ch * P:(ch + 1) * P]):
                w = float(weights[labels[j]])
                sel[il, ch * P + r] = w
                wc[r, ch] = w + NEG
                base = pbase[i] if clusters[i] == clusters[j] else qbase[i]
                mask = base.copy()
                mask[i] = False
                mask[j] = False
                madd[r, ch, :] = np.where(mask, 0.0, np.float32(NEG))
                madd[r, ch, j] = NEG
                jwin[r, ch, j - k0] = 1.0
        tables.append(dict(
            sel=sel, wc=wc,
            madd=madd.reshape(P, NCH * B).astype(ml_dtypes.float8_e5m2),
            jwin=jwin.reshape(P, NCH * KW).astype(ml_dtypes.float8_e5m2),
            anchors=np.array(core_anchors[c], np.int64)))
    return tables, NP, MA, kw0s


def _build_program(NP, MA, kw0s):
    NCH = NP // P
    NDC = D // P

    NWARM = 12

    XA = 2 * B
    XB = 2 * B
    MEGA_A = XA + NDC * MA + MA + NCH       # xtA | xmy | negones | wc
    F8LEN = NCH * B + NCH * KW + P          # madd | jwin | ident

    nc = bacc.Bacc("TRN2", target_bir_lowering=False, debug=False,
                   num_devices=NCORES)

    megaA = nc.dram_tensor("megaA", [P, MEGA_A], F32R, kind="ExternalInput")
    megaB = nc.dram_tensor("megaB", [P, XB], F32R, kind="ExternalInput")
    selA = nc.dram_tensor("selA", [MA, NP], F32R, kind="ExternalInput")
    megaF8 = nc.dram_tensor("megaF8", [P, F8LEN], F8, kind="ExternalInput")
    out_s = nc.dram_tensor("out_s", [P, 2 * NCH], F32, kind="ExternalOutput")

    with tile.TileContext(nc) as tc:
        with (
            tc.tile_pool(name="cst", bufs=1) as cst,
            tc.tile_pool(name="sm", bufs=8) as sm,
            tc.tile_pool(name="wps", bufs=1, space="PSUM") as wps,
            tc.tile_pool(name="vps", bufs=1, space="PSUM") as vps,
            tc.tile_pool(name="gps", bufs=6, space="PSUM") as gps,
        ):
            # ---- scratch for PE warmup (memset early, no input dep) ----
            scratch = cst.tile([P, 256], BF)
            nc.vector.memset(scratch[:], 0.25)

            # ---- 4 flat input DMAs, one per queue, needed-first ----
            mA = cst.tile([P, MEGA_A], F32R)
            nc.sync.dma_start(mA[:], megaA[:, :])
            mB = cst.tile([P, XB], F32R)
            nc.scalar.dma_start(mB[:], megaB[:, :])
            mF8 = cst.tile([P, F8LEN], F8)
            nc.gpsimd.dma_start(mF8[:], megaF8[:, :])
            sel_t = cst.tile([MA, NP], F32R)
            nc.scalar.dma_start(sel_t[:, 0:2 * P], selA[:, 0:2 * P])
            nc.scalar.dma_start(sel_t[:, 2 * P:], selA[:, 2 * P:])

            xt_c = [mA[:, c * B:(c + 1) * B] for c in range(2)] + \
                   [mB[:, c * B:(c + 1) * B] for c in range(2)]
            xmy = mA[:, XA:XA + NDC * MA]
            negones = mA[:, XA + NDC * MA:XA + NDC * MA + MA]
            wcbase = XA + NDC * MA + MA
            wc_t = mA[:, wcbase:wcbase + NCH].bitcast(F32)
            madd_t = mF8[:, 0:NCH * B]
            jwin_t = mF8[:, NCH * B:NCH * B + NCH * KW]
            ident_t = mF8[:, NCH * B + NCH * KW:]

            # ---- PE warmup to ramp the clock during DMA wait ----
            warm = wps.tile([P, 256], F32)
            for i in range(NWARM):
                nc.tensor.matmul(warm[:], lhsT=scratch[:, 0:P],
                                 rhs=scratch[:], start=True, stop=True)

            # ---- V = 2*Xa.X - ones*|x|^2 ;  xsq on Scalar/Pool ----
            v_psum = vps.tile([MA, B], F32)
            for dc in range(NDC):
                nc.tensor.matmul(v_psum[:],
                                 lhsT=xmy[:, dc * MA:(dc + 1) * MA],
                                 rhs=xt_c[dc], start=(dc == 0), stop=False)
                xsq = sm.tile([P, B], F32R, tag="xsq")
                if dc % 2 == 0:
                    nc.scalar.activation(
                        xsq[:], xt_c[dc], mybir.ActivationFunctionType.Square)
                else:
                    nc.vector.tensor_tensor(xsq[:], xt_c[dc], xt_c[dc],
                                            op=mybir.AluOpType.mult)
                nc.tensor.matmul(v_psum[:], lhsT=negones,
                                 rhs=xsq[:], start=False, stop=(dc == NDC - 1))

            for i in range(3):
                nc.tensor.matmul(warm[:], lhsT=scratch[:, 0:P],
                                 rhs=scratch[:], start=True, stop=True)
            v_sb = cst.tile([MA, B], F32R)
            nc.scalar.copy(v_sb[:], v_psum[:])

            # ---- per-chunk pipeline ----
            # column layout: S_c at 2c, count_c at 2c+1 (single out tensor)
            sacc_ch = cst.tile([P, 2 * NCH], F32)
            eS = cst.tile([P, NCH], F32)

            for c in range(NCH):
                vg = gps.tile([P, B], F32, tag="vg")
                nc.tensor.matmul(vg[:], lhsT=sel_t[:, c * P:(c + 1) * P],
                                 rhs=v_sb[:], start=True, stop=False)
                nc.tensor.matmul(vg[:], lhsT=ident_t,
                                 rhs=madd_t[:, c * B:(c + 1) * B],
                                 start=False, stop=True)
                # e = w*V_ij + NEG from the 64-col j-window (DVE)
                ej = sm.tile([P, KW], BF, tag="ej")
                k0 = kw0s[c]
                nc.vector.scalar_tensor_tensor(
                    ej[:], in0=jwin_t[:, c * KW:(c + 1) * KW], scalar=0.5,
                    in1=vg[:, k0:k0 + KW],
                    op0=mybir.AluOpType.is_gt, op1=mybir.AluOpType.mult,
                    accum_out=eS[:, c:c + 1])
                # cvec = (w + NEG) - e and its negation (Pool)
                cvec = sm.tile([P, 1], F32, tag="cv")
                nc.gpsimd.tensor_tensor(cvec[:], wc_t[:, c:c + 1],
                                        eS[:, c:c + 1],
                                        op=mybir.AluOpType.subtract)
                ncv = sm.tile([P, 1], F32, tag="ncv")
                nc.gpsimd.tensor_tensor(ncv[:], eS[:, c:c + 1],
                                        wc_t[:, c:c + 1],
                                        op=mybir.AluOpType.subtract)
                # relu+sum (ScalarE) and count (DVE) straight off PSUM
                zl = sm.tile([P, B], BF, tag="zl")
                nc.scalar.activation(zl[:], vg[:],
                                     mybir.ActivationFunctionType.Relu,
                                     bias=cvec[:, 0:1], scale=1.0,
                                     accum_out=sacc_ch[:, 2 * c:2 * c + 1])
                cl = sm.tile([P, B], BF, tag="cl")
                nc.vector.tensor_scalar(cl[:], vg[:], ncv[:, 0:1], None,
                                        op0=mybir.AluOpType.is_gt,
                                        op1=mybir.AluOpType.add,
                                        accum_out=sacc_ch[:, 2 * c + 1:2 * c + 2])
                if c == 6:
                    nc.sync.dma_start(out_s[:, 0:14], sacc_ch[:, 0:14])

            nc.sync.dma_start(out_s[:, 14:], sacc_ch[:, 14:])

    nc.compile()
    return nc


def _make_in_maps(embeddings, tables, NP, MA):
    x = np.ascontiguousarray(np.asarray(embeddings, dtype=np.float32))
    NCH = NP // P
    NDC = D // P
    xt = x.T  # [D, B]
    xtch = [np.ascontiguousarray(xt[dc * P:(dc + 1) * P, :])
            for dc in range(NDC)]
    in_maps = []
    for c in range(NCORES):
        t = tables[c]
        xmy = np.zeros((MA, D), np.float32)
        a = t["anchors"]
        xmy[:len(a)] = 2.0 * x[a]
        xmyT = xmy.T  # [D, MA]
        xmych = [xmyT[dc * P:(dc + 1) * P, :] for dc in range(NDC)]
        megaA = np.concatenate(
            [xtch[0], xtch[1]] + xmych +
            [np.full((P, MA), -1.0, np.float32), t["wc"]], axis=1)
        megaB = np.concatenate([xtch[2], xtch[3]], axis=1)
        megaF8 = np.concatenate(
            [t["madd"], t["jwin"],
             np.eye(P, dtype=ml_dtypes.float8_e5m2)], axis=1)
        in_maps.append({
            "megaA": np.ascontiguousarray(megaA),
            "megaB": np.ascontiguousarray(megaB),
            "selA": t["sel"],
            "megaF8": np.ascontiguousarray(megaF8),
        })
    return in_maps


def run(embeddings, labels, clusters, weights, trace=False):
    tables, NP, MA, kw0s = _host_prep(labels, clusters, weights)
    nc = _build_program(NP, MA, kw0s)
    in_maps = _make_in_maps(embeddings, tables, NP, MA)
    res = run_bass_kernel_spmd(nc, in_maps, core_ids=list(range(NCORES)),
                               trace=trace)
    S = 0.0
    C = 0.0
    for c, r in enumerate(res.results):
        o = np.asarray(r["out_s"], np.float64)
        S += float(o[:, 0::2].sum())
        C += float(o[:, 1::2].sum())
    loss = np.float32(np.float32(S) / np.float32(C + EPS))
    return np.asarray(loss, dtype=np.float32), res


def kernel(embeddings, labels, clusters, weights):
    loss, _ = run(embeddings, labels, clusters, weights)
    return loss



# revision 5
# speedup vs baseline: 1.1693x; 1.1693x over previous
"""Trainium2 Bass kernel for BatchAllTripletWithClustersLossSemiHard (v7).

Strategy (data-parallel over same-label pairs, 8 cores):
  Only (i,j) pairs with equal labels contribute.  The global pair list is
  built class-contiguously and split into 8 equal consecutive slices, so
  each core holds pairs from <=4 label classes.  Per core the k-axis is
  permuted so its own classes' columns come first: every excluded k
  (semi-hard rank parity, k==i, k==j) then lands in a fixed [0, WCAP)
  window, uniform across cores (SPMD program is identical; tables differ).

  Per pair-row p over permuted k:
      z[p,k] = V[i_p,k] + cvec_p + madd[p,k]
  with V[a,k] = w_a*(2 x_a.x_k - (|x_k|^2 - 512)) computed on device in
  bf16 (the 512 centering keeps bf16 rounding ~4x smaller), and
  cvec_p = w*(1 - 512 - V_ij) computed exactly on host (fp32).
  madd in {0, -4096} (fp8e5) covers only the [0, WCAP) window via a tiny
  ident x madd matmul; all other k are always valid.

  Device pipeline per chunk of 128 pairs: one bf16 one-hot matmul
  broadcasts V rows + accumulates, one fp8 matmul adds the mask window;
  relu+bias+row-sum alternates between ScalarE (activation w/ bias,
  accum_out) and DVE (tensor_scalar add+max, accum_out); counts come from
  DVE is_gt over the bf16 relu output (4x mode).  Raw [128, 2*NCH]
  partials are DMA'd out; the host does the final reduction.
"""

import numpy as np
import ml_dtypes

import concourse.bass as bass
import concourse.tile as tile
from concourse import bacc, mybir
from concourse.bass_utils import run_bass_kernel_spmd

EPS = 1e-8
NEG = -4096.0
COFF = 512.0  # |x|^2 centering offset
NCORES = 8
P = 128
F32 = mybir.dt.float32
BF = mybir.dt.bfloat16
F8 = mybir.dt.float8e5


def _host_prep(embeddings, labels, clusters, weights):
    x = np.ascontiguousarray(np.asarray(embeddings, dtype=np.float32))
    labels = np.asarray(labels).astype(np.int64)
    clusters = np.asarray(clusters).astype(np.int64)
    weights = np.asarray(weights).astype(np.float64)
    B, D = x.shape
    NDC = D // P

    leq = labels[None, :] == labels[:, None]
    rank = np.cumsum(leq.astype(np.int64), axis=1) - 1
    first = leq & (rank % 2 == 1)
    second = leq & (rank % 2 == 0)
    pbase = ~first
    qbase = ~second

    xd = x.astype(np.float64)
    sq = np.einsum("bd,bd->b", xd, xd)
    wper = weights[labels]

    # class-contiguous global pair list
    classes = [np.where(labels == g)[0] for g in range(int(labels.max()) + 1)]
    classes = [m for m in classes if len(m) > 0]
    all_pairs = []
    for gi, m in enumerate(classes):
        for i in m:
            for j in m:
                if j != i:
                    all_pairs.append((gi, int(i), int(j)))
    total = len(all_pairs)
    Q = (total + NCORES - 1) // NCORES
    NP = ((Q + P - 1) // P) * P
    NCH = NP // P

    cores = []
    for c in range(NCORES):
        pairs = all_pairs[c * Q:min((c + 1) * Q, total)]
        own_cls = sorted({g for g, i, j in pairs})
        own_cols = [int(k) for g in own_cls for k in classes[g]]
        anchors = sorted({i for g, i, j in pairs})
        cores.append(dict(pairs=pairs, own_cols=own_cols, anchors=anchors))
    MA = max(len(cc["anchors"]) for cc in cores)
    WCAP = ((max(len(cc["own_cols"]) for cc in cores) + 15) // 16) * 16
    assert MA <= P and WCAP <= B

    tables = []
    for c in range(NCORES):
        cc = cores[c]
        pairs, own_cols, anchors = cc["pairs"], cc["own_cols"], cc["anchors"]
        own_set = set(own_cols)
        perm = own_cols + [k for k in range(B) if k not in own_set]
        colpos = np.empty(B, np.int64)
        colpos[np.array(perm)] = np.arange(B)
        aidx = {i: a for a, i in enumerate(anchors)}

        sel = np.zeros((MA, NP), ml_dtypes.bfloat16)
        cvec = np.full((P, NCH), NEG, np.float32)
        madd = np.zeros((P, NCH * WCAP), ml_dtypes.float8_e5m2)
        for r, (g, i, j) in enumerate(pairs):
            ch, row = divmod(r, P)
            sel[aidx[i], ch * P + row] = 1.0
            vij = 2.0 * float(xd[i] @ xd[j]) - float(sq[j])
            cvec[row, ch] = np.float32(wper[i] * (1.0 - COFF - vij))
            base = pbase[i] if clusters[i] == clusters[j] else qbase[i]
            mask = base.copy()
            mask[i] = False
            mask[j] = False
            for k in classes[g]:
                if not mask[k]:
                    madd[row, ch * WCAP + colpos[k]] = NEG

        # xmy: [D, MA] = 2*w_a*x_a, D-chunked [128, NDC*MA]
        xmy = np.zeros((D, MA), np.float64)
        a_arr = np.array(anchors, np.int64)
        xmy[:, :len(anchors)] = (2.0 * wper[a_arr][None, :] * xd[a_arr].T)
        xmy = xmy.astype(ml_dtypes.bfloat16)
        megaM = np.concatenate([xmy[dc * P:(dc + 1) * P, :] for dc in range(NDC)],
                               axis=1)
        # X^T permuted, D-chunked [128, NDC*B]
        xt = x.T[:, np.array(perm)].astype(ml_dtypes.bfloat16)
        megaX = np.concatenate([xt[dc * P:(dc + 1) * P, :] for dc in range(NDC)],
                               axis=1)
        nrow = (sq[np.array(perm)] - COFF).astype(np.float32) \
            .astype(ml_dtypes.bfloat16)[None, :]
        negw = np.zeros((1, MA), ml_dtypes.bfloat16)
        negw[0, :len(anchors)] = (-wper[a_arr]).astype(ml_dtypes.bfloat16)
        tables.append(dict(
            megaX=np.ascontiguousarray(megaX),
            megaM=np.ascontiguousarray(megaM),
            selT=np.ascontiguousarray(sel),
            maddT=np.ascontiguousarray(madd),
            cvecT=np.ascontiguousarray(cvec),
            nrowT=np.ascontiguousarray(nrow),
            negwT=np.ascontiguousarray(negw),
            identT=np.eye(P, dtype=ml_dtypes.float8_e5m2),
        ))
    return tables, NP, MA, WCAP, B, NDC


def _build_program(NCH, MA, WCAP, B, NDC):
    NWARM = 10
    nc = bacc.Bacc("TRN2", target_bir_lowering=False, debug=False,
                   num_devices=NCORES)

    megaX = nc.dram_tensor("megaX", [P, NDC * B], BF, kind="ExternalInput")
    megaM = nc.dram_tensor("megaM", [P, NDC * MA], BF, kind="ExternalInput")
    selT = nc.dram_tensor("selT", [MA, NCH * P], BF, kind="ExternalInput")
    maddT = nc.dram_tensor("maddT", [P, NCH * WCAP], F8, kind="ExternalInput")
    cvecT = nc.dram_tensor("cvecT", [P, NCH], F32, kind="ExternalInput")
    nrowT = nc.dram_tensor("nrowT", [1, B], BF, kind="ExternalInput")
    negwT = nc.dram_tensor("negwT", [1, MA], BF, kind="ExternalInput")
    identT = nc.dram_tensor("identT", [P, P], F8, kind="ExternalInput")
    out_s = nc.dram_tensor("out_s", [P, 2 * NCH], F32, kind="ExternalOutput")

    with tile.TileContext(nc) as tc:
        with (
            tc.tile_pool(name="cst", bufs=1) as cst,
            tc.tile_pool(name="sm", bufs=4) as sm,
            tc.tile_pool(name="wps", bufs=1, space="PSUM") as wps,
            tc.tile_pool(name="vps", bufs=1, space="PSUM") as vps,
            tc.tile_pool(name="gps", bufs=6, space="PSUM") as gps,
        ):
            # scratch for PE warmup + ACT table preload (no input deps)
            scratch = cst.tile([P, 256], BF)
            nc.vector.memset(scratch[:], 0.25)
            zeros = cst.tile([P, B], BF)
            nc.gpsimd.memset(zeros[:], 0.0)
            zl = cst.tile([P, NCH * B], BF)
            sacc = cst.tile([P, 2 * NCH], F32)

            # input DMAs, needed-first per queue
            mX = cst.tile([P, NDC * B], BF)
            nc.sync.dma_start(mX[:, 0:2 * B], megaX[:, 0:2 * B])
            nc.sync.dma_start(mX[:, 2 * B:], megaX[:, 2 * B:])
            mM = cst.tile([P, NDC * MA], BF)
            nc.scalar.dma_start(mM[:], megaM[:, :])
            nrow_sb = cst.tile([1, B], BF)
            nc.scalar.dma_start(nrow_sb[:], nrowT[:, :])
            negw_sb = cst.tile([1, MA], BF)
            nc.scalar.dma_start(negw_sb[:], negwT[:, :])
            sel_sb = cst.tile([MA, NCH * P], BF)
            nc.scalar.dma_start(sel_sb[:], selT[:, :])
            cvec_sb = cst.tile([P, NCH], F32)
            nc.gpsimd.dma_start(cvec_sb[:], cvecT[:, :])
            madd_sb = cst.tile([P, NCH * WCAP], F8)
            nc.gpsimd.dma_start(madd_sb[:], maddT[:, :])
            ident_sb = cst.tile([P, P], F8)
            nc.gpsimd.dma_start(ident_sb[:], identT[:, :])

            # trigger ACT table load early + warm the PE clock
            tinya = sm.tile([P, 8], BF, tag="tinya")
            nc.scalar.activation(tinya[:], scratch[:, 0:8],
                                 mybir.ActivationFunctionType.Relu)
            warm = wps.tile([P, 256], F32)
            for _ in range(NWARM):
                nc.tensor.matmul(warm[:], lhsT=scratch[:, 0:P],
                                 rhs=scratch[:], start=True, stop=True)

            # V[a,k] = w_a*(2 x_a.x_k - (|x_k|^2 - 512)) in PSUM
            v_psum = vps.tile([MA, B], F32)
            for dc in range(NDC):
                nc.tensor.matmul(v_psum[:],
                                 lhsT=mM[:, dc * MA:(dc + 1) * MA],
                                 rhs=mX[:, dc * B:(dc + 1) * B],
                                 start=(dc == 0), stop=False)
            nc.tensor.matmul(v_psum[:], lhsT=negw_sb[0:1, :],
                             rhs=nrow_sb[0:1, :], start=False, stop=True)
            v_sb = cst.tile([MA, B], BF)
            nc.vector.tensor_copy(v_sb[:], v_psum[:])

            # per-chunk pipeline; sacc col 2c = row-sum, 2c+1 = count
            for c in range(NCH):
                vg = gps.tile([P, B], F32, tag="vg")
                nc.tensor.matmul(vg[:], lhsT=sel_sb[:, c * P:(c + 1) * P],
                                 rhs=v_sb[:], start=True, stop=False)
                nc.tensor.matmul(vg[:, 0:WCAP], lhsT=ident_sb[:],
                                 rhs=madd_sb[:, c * WCAP:(c + 1) * WCAP],
                                 start=False, stop=True)
                zc = zl[:, c * B:(c + 1) * B]
                if c % 2 == 0:
                    nc.scalar.activation(zc, vg[:],
                                         mybir.ActivationFunctionType.Relu,
                                         bias=cvec_sb[:, c:c + 1], scale=1.0,
                                         accum_out=sacc[:, 2 * c:2 * c + 1])
                else:
                    nc.vector.scalar_tensor_tensor(
                        zc, in0=vg[:], scalar=cvec_sb[:, c:c + 1],
                        in1=zeros[:], op0=mybir.AluOpType.add,
                        op1=mybir.AluOpType.max,
                        accum_out=sacc[:, 2 * c:2 * c + 1])
                cd = sm.tile([P, B], BF, tag="cd")
                nc.vector.tensor_scalar(cd[:], zc, float(EPS), None,
                                        op0=mybir.AluOpType.is_gt,
                                        op1=mybir.AluOpType.add,
                                        accum_out=sacc[:, 2 * c + 1:2 * c + 2])
                if c == NCH - 3:
                    nc.sync.dma_start(out_s[:, 0:2 * (NCH - 2)],
                                      sacc[:, 0:2 * (NCH - 2)])
            nc.sync.dma_start(out_s[:, 2 * (NCH - 2):],
                              sacc[:, 2 * (NCH - 2):])

    nc.compile()
    return nc


def run(embeddings, labels, clusters, weights, trace=False):
    tables, NP, MA, WCAP, B, NDC = _host_prep(embeddings, labels, clusters,
                                              weights)
    NCH = NP // P
    nc = _build_program(NCH, MA, WCAP, B, NDC)
    res = run_bass_kernel_spmd(nc, tables, core_ids=list(range(NCORES)),
                               trace=trace)
    S = 0.0
    C = 0.0
    for r in res.results:
        o = np.asarray(r["out_s"], np.float64)
        S += float(o[:, 0::2].sum())
        C += float(o[:, 1::2].sum())
    loss = np.float32(np.float32(S) / np.float32(C + EPS))
    return np.asarray(loss, dtype=np.float32), res


def kernel(embeddings, labels, clusters, weights):
    loss, _ = run(embeddings, labels, clusters, weights)
    return loss
